# revision 6
# baseline (speedup 1.0000x reference)
"""Distributed LSTM-cell kernel for one TRN2 chip (8 NeuronCores).

Problem: gates = w_ih @ x + b_ih + w_hh @ h_prev + b_hh   (4H x B)
         i,f,g,o = split(gates); c' = sig(f)*c + sig(i)*tanh(g)
         h' = sig(o)*tanh(c'); return sum(c' + h')

Sharding: tensor-parallel over the 4H gate dimension, interleaved so each
core owns rows [d*512,(d+1)*512) of EVERY gate (=> it owns h-rows
[d*512,(d+1)*512) of c'/h').  x / h_prev are replicated.  Each core emits
per-partition partial sums [128, 8]; the host reduces the 8*1024 partials.
No on-chip collective is needed.

Per-core compute: gates_d [2048, 1024] = W_d [2048, 8192] @ [x; h] [8192, 1024].

The final output is a near-cancelling sum (~31 out of 8.4M O(1) terms), so
matmul precision matters enormously: plain bf16 => ~15% rel err.  Schemes:
  bf16x3  split a=hi+lo (bf16); hi@hi + lo@hi + hi@lo      ~1e-4 rel err
  fp32 / fp32r: native fp32 matmul CRASHES the exec unit in this runtime
  and float32r returns garbage -- both unusable here (HW-verified).
  bf16x1/fp16x1  single-pass (accuracy reference only)
"""

import os

import numpy as np

D = 4096
H = 4096
B = 1024
NCORES = 8
RPC = 4 * H // NCORES // 4      # 512 rows per gate per core
HMT = RPC // 128                # 4 h-row tiles of 128 per core
KT = (D + H) // 128             # 64 contraction tiles
NN = B // 512                   # 2 batch halves
P = 128

SCHEME = os.environ.get("LSTM_SCHEME", "bf16x3")

_compiled = {}
LAST_RESULT = None


def _scheme_cfg(scheme):
    import concourse.mybir as mybir

    if scheme == "fp32":
        return dict(dt=mybir.dt.float32, nw=1, nx=1, pairs=[(0, 0)])
    if scheme == "fp32r":
        return dict(dt=mybir.dt.float32r, nw=1, nx=1, pairs=[(0, 0)])
    if scheme == "bf16x1":
        return dict(dt=mybir.dt.bfloat16, nw=1, nx=1, pairs=[(0, 0)])
    if scheme == "fp16x1":
        return dict(dt=mybir.dt.float16, nw=1, nx=1, pairs=[(0, 0)])
    if scheme == "bf16x3":
        return dict(
            dt=mybir.dt.bfloat16, nw=2, nx=2, pairs=[(0, 0), (1, 0), (0, 1)]
        )
    raise ValueError(scheme)


def _build(scheme):
    import concourse.bacc as bacc
    import concourse.mybir as mybir
    from concourse.tile import TileContext

    cfg = _scheme_cfg(scheme)
    dt_mm = cfg["dt"]
    pairs = cfg["pairs"]
    nsrc = len(pairs)
    f32 = mybir.dt.float32
    AFT = mybir.ActivationFunctionType
    ALU = mybir.AluOpType

    nc = bacc.Bacc("TRN2", target_bir_lowering=False, debug=False)

    w_drams = [
        nc.dram_tensor(f"wt{i}", [HMT, KT * P, 512], dt_mm, kind="ExternalInput")
        for i in range(cfg["nw"])
    ]
    x_drams = [
        nc.dram_tensor(f"xh{i}", [KT * P, B], dt_mm, kind="ExternalInput")
        for i in range(cfg["nx"])
    ]
    cprev_d = nc.dram_tensor("cprev", [RPC, B], f32, kind="ExternalInput")
    bias_d = nc.dram_tensor("bias", [P, HMT * 4], f32, kind="ExternalInput")
    out_d = nc.dram_tensor("out", [P, HMT * NN], f32, kind="ExternalOutput")

    with TileContext(nc) as tc:
        with (
            tc.tile_pool(name="wp", bufs=4) as wp,
            tc.tile_pool(name="xp", bufs=4) as xp,
            tc.tile_pool(name="pp", bufs=8, space="PSUM") as pp,
            tc.tile_pool(name="ep", bufs=2) as ep,
            tc.tile_pool(name="mp", bufs=1) as mp,
        ):
            bias_sb = mp.tile([P, HMT * 4], f32, tag="bias")
            nc.sync.dma_start(out=bias_sb[:, :], in_=bias_d[:, :])
            acc_sb = mp.tile([P, HMT * NN], f32, tag="acc")

            for hm in range(HMT):
                ps = [pp.tile([P, 512], f32, tag="ps", name=f"ps{hm}_{j}") for j in range(8)]
                for kt in range(KT):
                    wt = [
                        wp.tile([P, 512], dt_mm, tag=f"w{i}", name=f"w{i}_{hm}_{kt}")
                        for i in range(cfg["nw"])
                    ]
                    for i in range(cfg["nw"]):
                        nc.sync.dma_start(
                            out=wt[i][:, :],
                            in_=w_drams[i][hm, kt * P : (kt + 1) * P, :],
                        )
                    xt = [
                        xp.tile([P, B], dt_mm, tag=f"x{i}", name=f"x{i}_{hm}_{kt}")
                        for i in range(cfg["nx"])
                    ]
                    for i in range(cfg["nx"]):
                        nc.sync.dma_start(
                            out=xt[i][:, :],
                            in_=x_drams[i][kt * P : (kt + 1) * P, :],
                        )
                    for g in range(4):
                        for si, (wi, xi) in enumerate(pairs):
                            lhsT = wt[wi][:, g * P : (g + 1) * P]
                            for n in range(NN):
                                nc.tensor.matmul(
                                    ps[g * NN + n][:, :],
                                    lhsT,
                                    xt[xi][:, n * 512 : (n + 1) * 512],
                                    start=(kt == 0 and si == 0),
                                    stop=(kt == KT - 1 and si == nsrc - 1),
                                )

                for n in range(NN):
                    idx = hm * NN + n
                    cp = ep.tile([P, 512], f32, tag="cp")
                    nc.sync.dma_start(
                        out=cp[:, :],
                        in_=cprev_d[hm * P : (hm + 1) * P, n * 512 : (n + 1) * 512],
                    )
                    i_sb = ep.tile([P, 512], f32, tag="i")
                    f_sb = ep.tile([P, 512], f32, tag="f")
                    g_sb = ep.tile([P, 512], f32, tag="g")
                    o_sb = ep.tile([P, 512], f32, tag="o")
                    for t_sb, gi, fn in (
                        (i_sb, 0, AFT.Sigmoid),
                        (f_sb, 1, AFT.Sigmoid),
                        (g_sb, 2, AFT.Tanh),
                        (o_sb, 3, AFT.Sigmoid),
                    ):
                        nc.scalar.activation(
                            t_sb[:, :],
                            ps[gi * NN + n][:, :],
                            fn,
                            bias=bias_sb[:, hm * 4 + gi : hm * 4 + gi + 1],
                        )
                    t_fc = ep.tile([P, 512], f32, tag="fc")
                    nc.vector.tensor_mul(t_fc[:, :], f_sb[:, :], cp[:, :])
                    t_ig = ep.tile([P, 512], f32, tag="ig")
                    nc.vector.tensor_mul(t_ig[:, :], i_sb[:, :], g_sb[:, :])
                    t_c = ep.tile([P, 512], f32, tag="c")
                    nc.vector.tensor_add(t_c[:, :], t_fc[:, :], t_ig[:, :])
                    t_tc = ep.tile([P, 512], f32, tag="tc")
                    nc.scalar.activation(t_tc[:, :], t_c[:, :], AFT.Tanh)
                    t_h = ep.tile([P, 512], f32, tag="h")
                    nc.vector.tensor_mul(t_h[:, :], o_sb[:, :], t_tc[:, :])
                    t_s = ep.tile([P, 512], f32, tag="s")
                    nc.vector.tensor_add(t_s[:, :], t_c[:, :], t_h[:, :])
                    nc.vector.reduce_sum(
                        acc_sb[:, idx : idx + 1],
                        t_s[:, :],
                        axis=mybir.AxisListType.X,
                    )

            nc.sync.dma_start(out=out_d[:, :], in_=acc_sb[:, :])

    nc.compile()
    return nc


def _get_compiled(scheme):
    if scheme not in _compiled:
        _compiled[scheme] = _build(scheme)
    return _compiled[scheme]


def _split_lohi(a, np_dt):
    hi = a.astype(np_dt)
    lo = (a - hi.astype(np.float32)).astype(np_dt)
    return hi, lo


def _prep_inputs(scheme, x, h_prev, c_prev, w_ih, w_hh, b_ih, b_hh):
    import ml_dtypes

    f32 = np.float32
    x = np.asarray(x, f32)
    h_prev = np.asarray(h_prev, f32)
    c_prev = np.asarray(c_prev, f32)
    w_ih = np.asarray(w_ih, f32)
    w_hh = np.asarray(w_hh, f32)
    b = (np.asarray(b_ih, f32) + np.asarray(b_hh, f32)).reshape(4, NCORES, HMT, P)

    xh = np.concatenate([x, h_prev], axis=0)  # [8192, B]

    if scheme in ("fp32", "fp32r"):
        np_dt = f32
    elif scheme in ("bf16x1", "bf16x3"):
        np_dt = ml_dtypes.bfloat16
    elif scheme == "fp16x1":
        np_dt = np.float16
    else:
        raise ValueError(scheme)

    split = scheme.endswith("x3")
    if split:
        xh_hi, xh_lo = _split_lohi(xh, np_dt)
        x_maps = {"xh0": xh_hi, "xh1": xh_lo}
    else:
        x_maps = {"xh0": xh.astype(np_dt)}

    wih_r = w_ih.reshape(4, NCORES, RPC, D)
    whh_r = w_hh.reshape(4, NCORES, RPC, H)

    in_maps = []
    for d in range(NCORES):
        wc = np.concatenate([wih_r[:, d], whh_r[:, d]], axis=2)  # (4, 512, 8192)
        wc = wc.reshape(4, HMT, P, D + H)  # (g, hm, r, k)
        wt = np.ascontiguousarray(wc.transpose(1, 3, 0, 2)).reshape(
            HMT, D + H, 4 * P
        )  # (hm, k, g*128+r)
        m = dict(x_maps)
        if split:
            w_hi, w_lo = _split_lohi(wt, np_dt)
            m["wt0"] = w_hi
            m["wt1"] = w_lo
        else:
            m["wt0"] = wt.astype(np_dt)
        m["cprev"] = np.ascontiguousarray(c_prev[d * RPC : (d + 1) * RPC])
        m["bias"] = np.ascontiguousarray(
            b[:, d].transpose(2, 1, 0).reshape(P, HMT * 4)
        )
        in_maps.append(m)
    return in_maps


def _ensure_axon_ntff_hook():
    """Register the axon NTFF-profile hook if the container's `antenv` stub
    lacks `axon_hooks` (needed only for trace=True / BASS_TRACE runs)."""
    import contextlib
    import ctypes
    import sys
    import types

    try:
        from antenv import axon_hooks  # noqa: F401

        return
    except ImportError:
        pass
    try:
        import antenv
    except ImportError:
        return

    holder = {}
    mod = types.ModuleType("antenv.axon_hooks")
    mod.set_axon_ntff_profile_hook = lambda h: holder.__setitem__("h", h)
    mod.get_axon_ntff_profile_hook = lambda: holder.get("h")
    sys.modules["antenv.axon_hooks"] = mod
    antenv.axon_hooks = mod

    so_path = "/opt/axon/libaxon_pjrt.so"
    try:
        lib = ctypes.CDLL(so_path)
        if not hasattr(lib, "axon_start_nrt_profile"):
            return
        lib.axon_start_nrt_profile.argtypes = [
            ctypes.POINTER(ctypes.c_int64),
            ctypes.c_size_t,
        ]
        lib.axon_start_nrt_profile.restype = ctypes.c_int64
        lib.axon_stop_nrt_profile.argtypes = [ctypes.c_char_p]
        lib.axon_stop_nrt_profile.restype = ctypes.c_int64

        @contextlib.contextmanager
        def _hook(output_dir, device_ids):
            import jax

            jax.devices()
            if device_ids:
                ids = (ctypes.c_int64 * len(device_ids))(*device_ids)
                rc = lib.axon_start_nrt_profile(ids, len(device_ids))
            else:
                rc = lib.axon_start_nrt_profile(None, 0)
            if rc != 0:
                raise RuntimeError(f"axon_start_nrt_profile rc={rc}")
            try:
                yield
            finally:
                n = lib.axon_stop_nrt_profile(str(output_dir).encode())
                print(f"ntff profile: {n} file(s) -> {output_dir}", file=sys.stderr)

        mod.set_axon_ntff_profile_hook(_hook)
    except Exception:
        pass


def kernel(x, h_prev, c_prev, w_ih, w_hh, b_ih, b_hh):
    global LAST_RESULT
    from concourse.bass_utils import run_bass_kernel_spmd

    if os.environ.get("BASS_TRACE"):
        _ensure_axon_ntff_hook()

    scheme = SCHEME
    nc = _get_compiled(scheme)
    in_maps = _prep_inputs(scheme, x, h_prev, c_prev, w_ih, w_hh, b_ih, b_hh)
    res = run_bass_kernel_spmd(nc, in_maps, core_ids=list(range(NCORES)))
    LAST_RESULT = res
    total = np.float64(0.0)
    for r in res.results:
        total += np.asarray(r["out"], np.float64).sum()
    return np.array(total, dtype=np.float32)
